# revision 13
# baseline (speedup 1.0000x reference)
"""GRU-D Trainium2 Bass kernel.

Data-parallel over batch across 8 NeuronCores (32 rows/core). Per core:

Phase A (parallel over t): impute x, build x_cat^T = [x_imp; 1-m]^T per
timestep (via PE transposes) and the masked hidden-decay factor g_eff in
"X layout", staged to scratch DRAM.

Phase B (sequential scan over T=1024): per-step state kept in "X layout"
  X[32r+b, 32k+i] = S[b, 128k+32r+i]   (r,k in 0..3, b in 0..31, i in 0..31)
chosen so that (a) the three gate matmuls col-tile across the 4 PE column
groups with permuted weight tiles, landing results directly in X layout,
and (b) a single DVE 32x32 block-transpose of an X-layout tile yields the
transposed lhsT chunks needed for the next step's matmuls. The x-side
contributions (x_cat @ [W;V] + bias) accumulate into each gate's PSUM bank
ahead of the recurrent h-rounds, off the critical path.

Head: BatchNorm (eval) + decoder + log_softmax on-chip.
"""
import sys

sys.path.insert(0, "/opt/trn_rl_repo")

import numpy as np

import concourse.bass as bass
import concourse.bacc as bacc
import concourse.mybir as mybir
import concourse.tile as tile
from concourse.bass import ds
from concourse.bass_utils import run_bass_kernel_spmd

AF = mybir.ActivationFunctionType
ALU = mybir.AluOpType
F32 = mybir.dt.float32
BF16 = mybir.dt.bfloat16

B, T, F, H, O = 256, 1024, 128, 512, 64
NCORES = 8
N = B // NCORES  # 32 rows per core
BN_EPS = 1e-5


def build_kernel(nc, t_steps=T, scan_unroll=2, phasea_unroll=2):
    """Emit the full per-core program. Returns nothing; declares I/O tensors."""
    n_tiles = t_steps // 4  # phase A processes 4 timesteps (128 rows) per tile

    dt_in = [
        ("x_in", (N, t_steps, F)),
        ("delta_in", (N, t_steps, F)),
        ("m_in", (N, t_steps, F)),
        ("xf_in", (N, t_steps, F)),
        ("masks_ph", (128, n_tiles)),
        ("masks_sc", (128, t_steps)),
        ("wgx_rep", (128, F)),
        ("bgx_rep", (128, F)),
        ("wg_neg", (F, H)),
        ("g_bias_row", (1, H)),
        ("identity", (128, 128)),
        ("ones_col", (1, 128)),
        ("ones_bf", (1, 128)),
        ("wcat_perm", (3, 2, 4, 128, 128)),
        ("u_perm", (3, 4, 4, 128, 128)),
        ("bias_perm", (3, 4, 1, 128)),
        ("e4", (4, 128)),
        ("bnw_x", (128, 128)),
        ("bnb_x", (128, 128)),
        ("decw", (128, 4 * O)),
        ("decb_row", (1, O)),
    ]
    bf16_in = {"wg_neg", "g_bias_row", "ones_bf", "wcat_perm", "u_perm",
               "bias_perm", "e4"}
    d = {}
    for name, shape in dt_in:
        dt_ = BF16 if name in bf16_in else F32
        d[name] = nc.dram_tensor(name, shape, dt_, kind="ExternalInput").ap()

    out_logp = nc.dram_tensor("out_logp", (N, O), F32, kind="ExternalOutput").ap()
    out_hbn = nc.dram_tensor("out_hbn", (N, H), F32, kind="ExternalOutput").ap()

    # scratch in HBM
    xcatT = nc.dram_tensor("xcatT_scratch", (2, F, t_steps, N), BF16, kind="Internal").ap()
    geff = nc.dram_tensor("geff_scratch", (t_steps, N, H), F32, kind="Internal").ap()

    # input views for phase A: tile c covers timesteps 4c..4c+3, partitions (tp, b)
    def phase_a_src(ap):
        return ap.rearrange("b (c tp) f -> c tp b f", tp=4)

    x_src = phase_a_src(d["x_in"])
    dt_src = phase_a_src(d["delta_in"])
    m_src = phase_a_src(d["m_in"])
    xf_src = phase_a_src(d["xf_in"])

    # phase A store views
    xcatT_st = xcatT.rearrange("ch f (c tp) b -> ch c f (tp b)", tp=4)
    geff_st = geff.rearrange("(c tp) b h -> c (tp b) h", tp=4)

    # scan load views
    geff_ld = geff.rearrange("t b (k r2 i2) -> t r2 b k i2", k=4, r2=4)
    xcat_ld = xcatT.rearrange("ch f t b -> t f ch b")
    hbn_st = out_hbn.rearrange("b (k r2 i2) -> r2 b k i2", k=4, r2=4)

    with tile.TileContext(nc) as tc, \
            tc.tile_pool(name="const", bufs=1) as const:

        # ---- resident constants ----
        def load_const(name, shape=None, src=None):
            src = d[name] if src is None else src
            t_ = const.tile(list(shape or src.shape), src.dtype, tag=name)
            nc.sync.dma_start(t_, src)
            return t_

        wgx_sb = load_const("wgx_rep")
        bgx_sb = load_const("bgx_rep")
        wgneg_sb = load_const("wg_neg")
        gbias_sb = load_const("g_bias_row")
        ident_sb = load_const("identity")
        ones_sb = load_const("ones_col")
        onesbf_sb = load_const("ones_bf")
        mph_sb = load_const("masks_ph")
        msc_sb = load_const("masks_sc")
        wcat_sb = const.tile([128, 3 * 2 * 4 * 128], BF16, tag="wcat_perm")
        nc.sync.dma_start(
            wcat_sb.rearrange("p (g c r f) -> p g c r f", g=3, c=2, r=4),
            d["wcat_perm"].rearrange("g c r p f -> p g c r f"))
        u_sb = const.tile([128, 3 * 4 * 4 * 128], BF16, tag="u_perm")
        nc.sync.dma_start(
            u_sb.rearrange("p (g k r f) -> p g k r f", g=3, k=4, r=4),
            d["u_perm"].rearrange("g k r p f -> p g k r f"))
        bias_sb = const.tile([4, 3 * 128], BF16, tag="bias_perm")
        nc.sync.dma_start(
            bias_sb.rearrange("r (g f) -> r g f", g=3),
            d["bias_perm"].rearrange("g r p f -> r g (p f)"))
        e4_sb = load_const("e4")
        bnw_sb = load_const("bnw_x")
        bnb_sb = load_const("bnb_x")
        decw_sb = load_const("decw")
        decb_sb = load_const("decb_row")

        def wcat_t(g, c, r):
            o = ((g * 2 + c) * 4 + r) * 128
            return wcat_sb[:, o:o + 128]

        def u_t(g, k, r):
            o = ((g * 4 + k) * 4 + r) * 128
            return u_sb[:, o:o + 128]

        def bias_t(g):
            return bias_sb[:, g * 128:(g + 1) * 128]

        # 1 - masks_ph, for g_eff = mask*g + (1-mask)
        omph_sb = const.tile([128, n_tiles], F32, tag="omph")
        nc.vector.tensor_scalar(omph_sb, mph_sb, -1.0, 1.0, ALU.mult, ALU.add)

        # persistent scan state
        h_state = const.tile([128, 128], F32, tag="h_state")
        nc.vector.memset(h_state, 0.0)

        # ================= Phase A =================
        with tc.tile_pool(name="pa", bufs=6) as pa, \
                tc.tile_pool(name="pa_ps", bufs=2, space="PSUM") as pa_ps:

            def phase_a_body(cv):
                xt = pa.tile([128, F], F32, tag="xt")
                dtt = pa.tile([128, F], F32, tag="dtt")
                mt = pa.tile([128, F], F32, tag="mt")
                ft = pa.tile([128, F], F32, tag="ft")
                nc.sync.dma_start(xt, x_src[cv])
                nc.sync.dma_start(dtt, dt_src[cv])
                nc.sync.dma_start(mt, m_src[cv])
                nc.sync.dma_start(ft, xf_src[cv])

                # gamma_x = exp(-relu(delta*Wgx + bgx)) = min(exp(-(delta*Wgx+bgx)), 1)
                t1 = pa.tile([128, F], F32, tag="t1")
                nc.vector.tensor_tensor(t1, dtt, wgx_sb, ALU.mult)
                t2 = pa.tile([128, F], F32, tag="t2")
                nc.vector.tensor_tensor(t2, t1, bgx_sb, ALU.add)
                ex = pa.tile([128, F], F32, tag="ex")
                nc.scalar.activation(ex, t2, AF.Exp, scale=-1.0)
                gx = pa.tile([128, F], F32, tag="gx")
                nc.vector.tensor_scalar(gx, ex, 1.0, None, ALU.min)
                # x_rep = gx*(xf - 0.001) + 0.001 ; x_imp = m ? x_rep : x
                xr = pa.tile([128, F], F32, tag="xr")
                nc.vector.tensor_scalar(xr, ft, 0.001, None, ALU.subtract)
                xr2 = pa.tile([128, F], F32, tag="xr2")
                nc.vector.tensor_tensor(xr2, gx, xr, ALU.mult)
                xr3 = pa.tile([128, F], F32, tag="xr3")
                nc.vector.tensor_scalar(xr3, xr2, 0.001, None, ALU.add)
                # x_imp = x + m*(x_rep - x)   (m is exactly 0/1)
                xi1 = pa.tile([128, F], F32, tag="xi1")
                nc.vector.tensor_tensor(xi1, xr3, xt, ALU.subtract)
                xi2 = pa.tile([128, F], F32, tag="xi2")
                nc.vector.tensor_tensor(xi2, mt, xi1, ALU.mult)
                ximp = pa.tile([128, F], F32, tag="ximp")
                nc.vector.tensor_tensor(ximp, xt, xi2, ALU.add)
                onem = pa.tile([128, F], F32, tag="onem")
                nc.vector.tensor_scalar(onem, mt, -1.0, 1.0, ALU.mult, ALU.add)

                # transposes to [F, rows]
                ps1 = pa_ps.tile([128, 128], F32, tag="ps1")
                nc.tensor.transpose(ps1, ximp, ident_sb)
                ps2 = pa_ps.tile([128, 128], F32, tag="ps2")
                nc.tensor.transpose(ps2, onem, ident_sb)
                ximpT = pa.tile([128, 128], BF16, tag="ximpT")
                nc.scalar.copy(ximpT, ps1)
                onemT = pa.tile([128, 128], BF16, tag="onemT")
                nc.scalar.copy(onemT, ps2)

                # g_eff: gamma_h = exp(-relu(m@Wg + bg)); m@Wg = one_m@(-Wg) + colsum(Wg)
                psg = pa_ps.tile([128, H], F32, tag="psg")
                nc.tensor.matmul(psg, onemT, wgneg_sb, start=True, stop=False)
                nc.tensor.matmul(psg, onesbf_sb, gbias_sb, start=False, stop=True)
                eg = pa.tile([128, H], F32, tag="eg")
                nc.scalar.activation(eg, psg, AF.Exp, scale=-1.0)
                gm = pa.tile([128, H], F32, tag="gm")
                nc.vector.tensor_scalar(gm, eg, 1.0, None, ALU.min)
                ge = pa.tile([128, H], F32, tag="ge")
                nc.vector.tensor_scalar(ge, gm, mph_sb[:, ds(cv, 1)],
                                        omph_sb[:, ds(cv, 1)], ALU.mult, ALU.add)

                nc.sync.dma_start(xcatT_st[0, cv], ximpT)
                nc.sync.dma_start(xcatT_st[1, cv], onemT)
                nc.sync.dma_start(geff_st[cv], ge)

            with tc.For_i(0, n_tiles, phasea_unroll) as cv0:
                for u in range(phasea_unroll):
                    phase_a_body(cv0 + u)

        # ================= Phase B: the scan =================
        with tc.tile_pool(name="pb", bufs=4) as pb, \
                tc.tile_pool(name="pb_ps", bufs=2, space="PSUM") as pb_ps:

            def scan_body(iv):
                g_t = pb.tile([128, 128], F32, tag="g_t")
                nc.sync.dma_start(g_t, geff_ld[ds(iv, 1)])
                xct = pb.tile([128, 2 * N], BF16, tag="xct")
                nc.sync.dma_start(xct, xcat_ld[ds(iv, 1)])

                hd = pb.tile([128, 128], F32, tag="hd")
                nc.vector.tensor_tensor(hd, g_t, h_state, ALU.mult)
                hd_bf = pb.tile([128, 128], BF16, tag="hd_bf")
                nc.vector.tensor_copy(hd_bf, hd)
                hdT = pb.tile([128, 128], BF16, tag="hdT")
                nc.vector.transpose(hdT, hd_bf)

                GZ, GR, GC = 0, 1, 2
                ps = {}
                for g, tag in ((GR, "psr"), (GZ, "psz"), (GC, "psc")):
                    ps[g] = pb_ps.tile([128, 128], F32, tag=tag, name=tag)

                # bias broadcast: one full-partition start=True MM per bank
                # (E4.T @ bias_rows == bias in X layout; rank-4 trick)
                for g in (GR, GZ, GC):
                    nc.tensor.matmul(ps[g], e4_sb, bias_t(g),
                                     start=True, stop=False,
                                     skip_group_check=True)
                # x-side rounds (independent of h)
                for g in (GR, GZ, GC):
                    for r4 in range(4):
                        for c in range(2):
                            nc.tensor.matmul(
                                ps[g][32 * r4:32 * r4 + 32, :],
                                xct[:, c * N:(c + 1) * N],
                                wcat_t(g, c, r4),
                                start=False, stop=False,
                                tile_position=(0, 32 * r4),
                                skip_group_check=True)
                # recurrent rounds for r and z gates
                for g in (GR, GZ):
                    for r4 in range(4):
                        for k in range(4):
                            nc.tensor.matmul(
                                ps[g][32 * r4:32 * r4 + 32, :],
                                hdT[:, 32 * k:32 * k + 32],
                                u_t(g, k, r4),
                                start=False, stop=(k == 3),
                                tile_position=(0, 32 * r4),
                                skip_group_check=True)

                r_sb = pb.tile([128, 128], F32, tag="r_sb")
                nc.scalar.activation(r_sb, ps[GR], AF.Sigmoid)
                z_sb = pb.tile([128, 128], F32, tag="z_sb")
                nc.scalar.activation(z_sb, ps[GZ], AF.Sigmoid)

                hr = pb.tile([128, 128], BF16, tag="hr")
                nc.vector.tensor_tensor(hr, hd, r_sb, ALU.mult)
                hrT = pb.tile([128, 128], BF16, tag="hrT")
                nc.vector.transpose(hrT, hr)

                for r4 in range(4):
                    for k in range(4):
                        nc.tensor.matmul(
                            ps[GC][32 * r4:32 * r4 + 32, :],
                            hrT[:, 32 * k:32 * k + 32],
                            u_t(GC, k, r4),
                            start=False, stop=(k == 3),
                            tile_position=(0, 32 * r4),
                            skip_group_check=True)

                ht = pb.tile([128, 128], F32, tag="ht")
                nc.scalar.activation(ht, ps[GC], AF.Tanh)

                # h_new = hd + (z*mask) * (ht - hd)
                zeff = pb.tile([128, 128], F32, tag="zeff")
                nc.vector.tensor_scalar(zeff, z_sb, msc_sb[:, ds(iv, 1)], None, ALU.mult)
                dlt = pb.tile([128, 128], F32, tag="dlt")
                nc.vector.tensor_tensor(dlt, ht, hd, ALU.subtract)
                uu = pb.tile([128, 128], F32, tag="uu")
                nc.vector.tensor_tensor(uu, zeff, dlt, ALU.mult)
                nc.vector.tensor_tensor(h_state, hd, uu, ALU.add)

            with tc.For_i(0, t_steps, scan_unroll,
                          hint_engines=(mybir.EngineType.PE,)) as iv0:
                for u in range(scan_unroll):
                    scan_body(iv0 + u)

        # ================= Head =================
        with tc.tile_pool(name="ph", bufs=2) as ph, \
                tc.tile_pool(name="ph_ps", bufs=2, space="PSUM") as ph_ps:
            hbn0 = ph.tile([128, 128], F32, tag="hbn0")
            nc.vector.tensor_tensor(hbn0, h_state, bnw_sb, ALU.mult)
            hbn = ph.tile([128, 128], F32, tag="hbn")
            nc.vector.tensor_tensor(hbn, hbn0, bnb_sb, ALU.add)
            nc.sync.dma_start(hbn_st, hbn)

            hbnT = ph.tile([128, 128], F32, tag="hbnT")
            nc.vector.transpose(hbnT, hbn)
            psd = ph_ps.tile([N, O], F32, tag="psd")
            for k in range(4):
                nc.tensor.matmul(psd, hbnT[:, 32 * k:32 * k + 32],
                                 decw_sb[:, k * O:(k + 1) * O],
                                 start=(k == 0), stop=False)
            nc.tensor.matmul(psd, ones_sb[:, :N], decb_sb,
                             start=False, stop=True)

            negmax = ph.tile([N, 1], F32, tag="negmax")
            nc.vector.tensor_reduce(negmax, psd, mybir.AxisListType.X, ALU.max,
                                    negate=True)
            esum = ph.tile([N, 1], F32, tag="esum")
            etile = ph.tile([N, O], F32, tag="etile")
            nc.scalar.activation(etile, psd, AF.Exp, bias=negmax, accum_out=esum)
            lns = ph.tile([N, 1], F32, tag="lns")
            nc.scalar.activation(lns, esum, AF.Ln)
            nlns = ph.tile([N, 1], F32, tag="nlns")
            nc.vector.tensor_scalar(nlns, lns, -1.0, None, ALU.mult)
            lp = ph.tile([N, O], F32, tag="lp")
            nc.vector.tensor_scalar(lp, psd, negmax, nlns, ALU.add, ALU.add)
            nc.sync.dma_start(out_logp, lp)


# ---------------- host side ----------------

def _perm_cols(W, r):
    cols = np.concatenate(
        [np.arange(128 * k + 32 * r, 128 * k + 32 * r + 32) for k in range(4)])
    return np.ascontiguousarray(W[:, cols])


def _to_x_bcast(v):
    """per-h vector [H] -> X-layout [128,128] tile (same for every b)"""
    out = np.zeros((128, 128), np.float32)
    for r in range(4):
        for k in range(4):
            out[32 * r:32 * r + 32, 32 * k:32 * k + 32] = \
                v[128 * k + 32 * r:128 * k + 32 * r + 32][None, :]
    return out


def prep_host_inputs(inputs, t_steps=T, ncores=NCORES):
    """Build per-core in_maps (numpy only: slicing/layout/permutation)."""
    f32 = np.float32
    x = np.asarray(inputs["x"], f32)
    delta = np.asarray(inputs["delta"], f32)
    m = np.asarray(inputs["m"], f32)
    xf = np.asarray(inputs["x_forward"], f32)
    bs = np.asarray(inputs["batch_sizes"])
    n_tiles = t_steps // 4

    W_r, U_r, V_r, b_r = (np.asarray(inputs[k], f32) for k in ("W_r", "U_r", "V_r", "b_r"))
    W_z, U_z, V_z, b_z = (np.asarray(inputs[k], f32) for k in ("W_z", "U_z", "V_z", "b_z"))
    W, U, V, b = (np.asarray(inputs[k], f32) for k in ("W", "U", "V", "b"))
    Wgx, bgx = np.asarray(inputs["W_gamma_x"], f32), np.asarray(inputs["b_gamma_x"], f32)
    Wgh, bgh = np.asarray(inputs["W_gamma_h"], f32), np.asarray(inputs["b_gamma_h"], f32)
    decW, decb = np.asarray(inputs["dec_W"], f32), np.asarray(inputs["dec_b"], f32)
    bnw, bnb = np.asarray(inputs["bn_w"], f32), np.asarray(inputs["bn_b"], f32)

    gates = [(W_z, V_z, U_z, b_z), (W_r, V_r, U_r, b_r), (W, V, U, b)]
    wcat_perm = np.zeros((3, 2, 4, 128, 128), f32)
    u_perm = np.zeros((3, 4, 4, 128, 128), f32)
    bias_perm = np.zeros((3, 4, 1, 128), f32)
    for g, (Wg_, Vg_, Ug_, bg_) in enumerate(gates):
        Wc = np.vstack([Wg_, Vg_])  # [256, H]
        for r in range(4):
            pc = _perm_cols(Wc, r)
            wcat_perm[g, 0, r] = pc[:128]
            wcat_perm[g, 1, r] = pc[128:]
            bias_perm[g, r, 0] = _perm_cols(bg_[None, :], r)[0]
            pu = _perm_cols(Ug_, r)
            for k in range(4):
                u_perm[g, k, r] = pu[128 * k:128 * k + 128]

    bn_scale = bnw / np.sqrt(1.0 + BN_EPS)
    decw_packed = np.zeros((128, 4 * O), f32)
    for k in range(4):
        decw_packed[:, k * O:(k + 1) * O] = decW[128 * k:128 * k + 128, :]

    import ml_dtypes
    bf16 = ml_dtypes.bfloat16
    shared = {
        "wgx_rep": np.tile(Wgx[None, :], (128, 1)).astype(f32),
        "bgx_rep": np.tile(bgx[None, :], (128, 1)).astype(f32),
        "wg_neg": (-Wgh).astype(bf16),
        "g_bias_row": (Wgh.sum(0) + bgh)[None, :].astype(bf16),
        "identity": np.eye(128, dtype=f32),
        "e4": np.repeat(np.eye(4, dtype=bf16), 32, axis=1),
        "ones_col": np.ones((1, 128), f32),
        "ones_bf": np.ones((1, 128), bf16),
        "wcat_perm": wcat_perm.astype(bf16),
        "u_perm": u_perm.astype(bf16),
        "bias_perm": bias_perm.astype(bf16),
        "bnw_x": _to_x_bcast(bn_scale),
        "bnb_x": _to_x_bcast(bnb),
        "decw": decw_packed,
        "decb_row": decb[None, :].astype(f32),
    }

    in_maps = []
    n = B // ncores
    for c in range(ncores):
        sl = slice(c * n, (c + 1) * n)
        rows = np.arange(c * n, (c + 1) * n)
        masks = (rows[None, :] < bs[:t_steps, None]).astype(f32)  # [T, n]
        masks_ph = np.zeros((128, n_tiles), f32)
        masks_sc = np.zeros((128, t_steps), f32)
        for tp in range(4):
            masks_ph[32 * tp:32 * tp + 32, :] = masks[np.arange(n_tiles) * 4 + tp].T
        for r in range(4):
            masks_sc[32 * r:32 * r + 32, :] = masks.T
        im = dict(shared)
        im.update({
            "x_in": np.ascontiguousarray(x[sl, :t_steps]),
            "delta_in": np.ascontiguousarray(delta[sl, :t_steps]),
            "m_in": np.ascontiguousarray(m[sl, :t_steps]),
            "xf_in": np.ascontiguousarray(xf[sl, :t_steps]),
            "masks_ph": masks_ph,
            "masks_sc": masks_sc,
        })
        in_maps.append(im)
    return in_maps


_CACHED = {}


def _build(t_steps=T, scan_unroll=2, phasea_unroll=2):
    key = (t_steps, scan_unroll, phasea_unroll)
    if key not in _CACHED:
        nc = bacc.Bacc("TRN2", target_bir_lowering=False, debug=False)
        build_kernel(nc, t_steps=t_steps, scan_unroll=scan_unroll,
                     phasea_unroll=phasea_unroll)
        nc.compile()
        _CACHED[key] = nc
    return _CACHED[key]


def kernel(**inputs):
    nc = _build()
    in_maps = prep_host_inputs(inputs)
    res = run_bass_kernel_spmd(nc, in_maps, core_ids=list(range(NCORES)))
    outs = res.results
    logp = np.concatenate([outs[c]["out_logp"] for c in range(NCORES)], axis=0)
    hbn = np.concatenate([outs[c]["out_hbn"] for c in range(NCORES)], axis=0)
    return logp.astype(np.float32), hbn.astype(np.float32)


# revision 14
# speedup vs baseline: 1.0325x; 1.0325x over previous
"""GRU-D Trainium2 Bass kernel.

Data-parallel over batch across 8 NeuronCores (32 rows/core). Per core:

Phase A (parallel over t): impute x, build x_cat^T = [x_imp; 1-m]^T per
timestep (via PE transposes) and the masked hidden-decay factor g_eff in
"X layout", staged to scratch DRAM.

Phase B (sequential scan over T=1024): per-step state kept in "X layout"
  X[32r+b, 32k+i] = S[b, 128k+32r+i]   (r,k in 0..3, b in 0..31, i in 0..31)
chosen so that (a) the three gate matmuls col-tile across the 4 PE column
groups with permuted weight tiles, landing results directly in X layout,
and (b) a single DVE 32x32 block-transpose of an X-layout tile yields the
transposed lhsT chunks needed for the next step's matmuls. The x-side
contributions (x_cat @ [W;V] + bias) accumulate into each gate's PSUM bank
ahead of the recurrent h-rounds, off the critical path.

Head: BatchNorm (eval) + decoder + log_softmax on-chip.
"""
import sys

sys.path.insert(0, "/opt/trn_rl_repo")

import numpy as np

import concourse.bass as bass
import concourse.bacc as bacc
import concourse.mybir as mybir
import concourse.tile as tile
from concourse.bass import ds
from concourse.bass_utils import run_bass_kernel_spmd

AF = mybir.ActivationFunctionType
ALU = mybir.AluOpType
F32 = mybir.dt.float32
BF16 = mybir.dt.bfloat16

B, T, F, H, O = 256, 1024, 128, 512, 64
NCORES = 8
N = B // NCORES  # 32 rows per core
BN_EPS = 1e-5


def build_kernel(nc, t_steps=T, scan_unroll=4, phasea_unroll=4):
    """Emit the full per-core program. Returns nothing; declares I/O tensors."""
    n_tiles = t_steps // 4  # phase A processes 4 timesteps (128 rows) per tile

    dt_in = [
        ("x_in", (N, t_steps, F)),
        ("delta_in", (N, t_steps, F)),
        ("m_in", (N, t_steps, F)),
        ("xf_in", (N, t_steps, F)),
        ("masks_ph", (128, n_tiles)),
        ("masks_sc", (128, t_steps)),
        ("wgx_rep", (128, F)),
        ("bgx_rep", (128, F)),
        ("wg_neg", (F, H)),
        ("g_bias_row", (1, H)),
        ("identity", (128, 128)),
        ("ones_col", (1, 128)),
        ("ones_bf", (1, 128)),
        ("wcat_perm", (3, 2, 4, 128, 128)),
        ("u_perm", (3, 4, 4, 128, 128)),
        ("bias_perm", (3, 4, 1, 128)),
        ("e4", (4, 128)),
        ("bnw_x", (128, 128)),
        ("bnb_x", (128, 128)),
        ("decw", (128, 4 * O)),
        ("decb_row", (1, O)),
    ]
    bf16_in = {"wg_neg", "g_bias_row", "ones_bf", "wcat_perm", "u_perm",
               "bias_perm", "e4"}
    d = {}
    for name, shape in dt_in:
        dt_ = BF16 if name in bf16_in else F32
        d[name] = nc.dram_tensor(name, shape, dt_, kind="ExternalInput").ap()

    out_logp = nc.dram_tensor("out_logp", (N, O), F32, kind="ExternalOutput").ap()
    out_hbn = nc.dram_tensor("out_hbn", (N, H), F32, kind="ExternalOutput").ap()

    # scratch in HBM
    xcatT = nc.dram_tensor("xcatT_scratch", (2, F, t_steps, N), BF16, kind="Internal").ap()
    geff = nc.dram_tensor("geff_scratch", (t_steps, N, H), F32, kind="Internal").ap()

    # input views for phase A: tile c covers timesteps 4c..4c+3, partitions (tp, b)
    def phase_a_src(ap):
        return ap.rearrange("b (c tp) f -> c tp b f", tp=4)

    x_src = phase_a_src(d["x_in"])
    dt_src = phase_a_src(d["delta_in"])
    m_src = phase_a_src(d["m_in"])
    xf_src = phase_a_src(d["xf_in"])

    # phase A store views
    xcatT_st = xcatT.rearrange("ch f (c tp) b -> ch c f (tp b)", tp=4)
    geff_st = geff.rearrange("(c tp) b h -> c (tp b) h", tp=4)

    # scan load views
    geff_ld = geff.rearrange("t b (k r2 i2) -> t r2 b k i2", k=4, r2=4)
    xcat_ld = xcatT.rearrange("ch f t b -> t f ch b")
    hbn_st = out_hbn.rearrange("b (k r2 i2) -> r2 b k i2", k=4, r2=4)

    with tile.TileContext(nc) as tc, \
            tc.tile_pool(name="const", bufs=1) as const:

        # ---- resident constants ----
        def load_const(name, shape=None, src=None):
            src = d[name] if src is None else src
            t_ = const.tile(list(shape or src.shape), src.dtype, tag=name)
            nc.sync.dma_start(t_, src)
            return t_

        wgx_sb = load_const("wgx_rep")
        bgx_sb = load_const("bgx_rep")
        wgneg_sb = load_const("wg_neg")
        gbias_sb = load_const("g_bias_row")
        ident_sb = load_const("identity")
        ones_sb = load_const("ones_col")
        onesbf_sb = load_const("ones_bf")
        mph_sb = load_const("masks_ph")
        msc_sb = load_const("masks_sc")
        wcat_sb = const.tile([128, 3 * 2 * 4 * 128], BF16, tag="wcat_perm")
        nc.sync.dma_start(
            wcat_sb.rearrange("p (g c r f) -> p g c r f", g=3, c=2, r=4),
            d["wcat_perm"].rearrange("g c r p f -> p g c r f"))
        u_sb = const.tile([128, 3 * 4 * 4 * 128], BF16, tag="u_perm")
        nc.sync.dma_start(
            u_sb.rearrange("p (g k r f) -> p g k r f", g=3, k=4, r=4),
            d["u_perm"].rearrange("g k r p f -> p g k r f"))
        bias_sb = const.tile([4, 3 * 128], BF16, tag="bias_perm")
        nc.sync.dma_start(
            bias_sb.rearrange("r (g f) -> r g f", g=3),
            d["bias_perm"].rearrange("g r p f -> r g (p f)"))
        e4_sb = load_const("e4")
        bnw_sb = load_const("bnw_x")
        bnb_sb = load_const("bnb_x")
        decw_sb = load_const("decw")
        decb_sb = load_const("decb_row")

        def wcat_t(g, c, r):
            o = ((g * 2 + c) * 4 + r) * 128
            return wcat_sb[:, o:o + 128]

        def u_t(g, k, r):
            o = ((g * 4 + k) * 4 + r) * 128
            return u_sb[:, o:o + 128]

        def bias_t(g):
            return bias_sb[:, g * 128:(g + 1) * 128]

        # 1 - masks_ph, for g_eff = mask*g + (1-mask)
        omph_sb = const.tile([128, n_tiles], F32, tag="omph")
        nc.vector.tensor_scalar(omph_sb, mph_sb, -1.0, 1.0, ALU.mult, ALU.add)

        # persistent scan state
        h_state = const.tile([128, 128], F32, tag="h_state")
        nc.vector.memset(h_state, 0.0)

        # ================= Phase A =================
        with tc.tile_pool(name="pa", bufs=6) as pa, \
                tc.tile_pool(name="pa_ps", bufs=2, space="PSUM") as pa_ps:

            def phase_a_body(cv):
                xt = pa.tile([128, F], F32, tag="xt")
                dtt = pa.tile([128, F], F32, tag="dtt")
                mt = pa.tile([128, F], F32, tag="mt")
                ft = pa.tile([128, F], F32, tag="ft")
                nc.sync.dma_start(xt, x_src[cv])
                nc.scalar.dma_start(dtt, dt_src[cv])
                nc.gpsimd.dma_start(mt, m_src[cv])
                nc.sync.dma_start(ft, xf_src[cv])

                # gamma_x = exp(-relu(delta*Wgx + bgx)) = min(exp(-(delta*Wgx+bgx)), 1)
                t1 = pa.tile([128, F], F32, tag="t1")
                nc.vector.tensor_tensor(t1, dtt, wgx_sb, ALU.mult)
                t2 = pa.tile([128, F], F32, tag="t2")
                nc.vector.tensor_tensor(t2, t1, bgx_sb, ALU.add)
                ex = pa.tile([128, F], F32, tag="ex")
                nc.scalar.activation(ex, t2, AF.Relu)
                gx = pa.tile([128, F], F32, tag="gx")
                nc.scalar.activation(gx, ex, AF.Exp, scale=-1.0)
                # x_rep = gx*(xf - 0.001) + 0.001 ; x_imp = m ? x_rep : x
                xr = pa.tile([128, F], F32, tag="xr")
                nc.vector.tensor_scalar(xr, ft, 0.001, None, ALU.subtract)
                xr2 = pa.tile([128, F], F32, tag="xr2")
                nc.vector.tensor_tensor(xr2, gx, xr, ALU.mult)
                xr3 = pa.tile([128, F], F32, tag="xr3")
                nc.vector.tensor_scalar(xr3, xr2, 0.001, None, ALU.add)
                # x_imp = x + m*(x_rep - x)   (m is exactly 0/1)
                xi1 = pa.tile([128, F], F32, tag="xi1")
                nc.vector.tensor_tensor(xi1, xr3, xt, ALU.subtract)
                xi2 = pa.tile([128, F], F32, tag="xi2")
                nc.vector.tensor_tensor(xi2, mt, xi1, ALU.mult)
                ximp = pa.tile([128, F], F32, tag="ximp")
                nc.vector.tensor_tensor(ximp, xt, xi2, ALU.add)
                onem = pa.tile([128, F], F32, tag="onem")
                nc.vector.tensor_scalar(onem, mt, -1.0, 1.0, ALU.mult, ALU.add)

                # transposes to [F, rows]
                ps1 = pa_ps.tile([128, 128], F32, tag="ps1")
                nc.tensor.transpose(ps1, ximp, ident_sb)
                ps2 = pa_ps.tile([128, 128], F32, tag="ps2")
                nc.tensor.transpose(ps2, onem, ident_sb)
                ximpT = pa.tile([128, 128], BF16, tag="ximpT")
                nc.scalar.copy(ximpT, ps1)
                onemT = pa.tile([128, 128], BF16, tag="onemT")
                nc.scalar.copy(onemT, ps2)

                # g_eff: gamma_h = exp(-relu(m@Wg + bg)); m@Wg = one_m@(-Wg) + colsum(Wg)
                psg = pa_ps.tile([128, H], F32, tag="psg")
                nc.tensor.matmul(psg, onemT, wgneg_sb, start=True, stop=False)
                nc.tensor.matmul(psg, onesbf_sb, gbias_sb, start=False, stop=True)
                eg = pa.tile([128, H], F32, tag="eg")
                nc.scalar.activation(eg, psg, AF.Relu)
                gm = pa.tile([128, H], F32, tag="gm")
                nc.scalar.activation(gm, eg, AF.Exp, scale=-1.0)
                ge = pa.tile([128, H], F32, tag="ge")
                nc.vector.tensor_scalar(ge, gm, mph_sb[:, ds(cv, 1)],
                                        omph_sb[:, ds(cv, 1)], ALU.mult, ALU.add)

                nc.sync.dma_start(xcatT_st[0, cv], ximpT)
                nc.scalar.dma_start(xcatT_st[1, cv], onemT)
                nc.gpsimd.dma_start(geff_st[cv], ge)

            with tc.For_i(0, n_tiles, phasea_unroll) as cv0:
                for u in range(phasea_unroll):
                    phase_a_body(cv0 + u)

        # ================= Phase B: the scan =================
        with tc.tile_pool(name="pb", bufs=4) as pb, \
                tc.tile_pool(name="pb_ps", bufs=2, space="PSUM") as pb_ps:

            def scan_body(iv):
                # split small-descriptor loads across DMA queues/engines
                g_t = pb.tile([128, 128], F32, tag="g_t")
                g_src = geff_ld[ds(iv, 1)]
                nc.sync.dma_start(g_t[0:32, :], g_src[:, 0])
                nc.scalar.dma_start(g_t[32:64, :], g_src[:, 1])
                nc.gpsimd.dma_start(g_t[64:96, :], g_src[:, 2])
                nc.gpsimd.dma_start(g_t[96:128, :], g_src[:, 3])
                xct = pb.tile([128, 2 * N], BF16, tag="xct")
                x_src_t = xcat_ld[ds(iv, 1)]
                nc.sync.dma_start(xct[:, 0:N], x_src_t[:, :, 0])
                nc.scalar.dma_start(xct[:, N:2 * N], x_src_t[:, :, 1])

                hd = pb.tile([128, 128], F32, tag="hd")
                nc.vector.tensor_tensor(hd, g_t, h_state, ALU.mult)
                hd_bf = pb.tile([128, 128], BF16, tag="hd_bf")
                nc.vector.tensor_copy(hd_bf, hd)
                hdT = pb.tile([128, 128], BF16, tag="hdT")
                nc.vector.transpose(hdT, hd_bf)

                GZ, GR, GC = 0, 1, 2
                ps = {}
                for g, tag in ((GR, "psr"), (GZ, "psz"), (GC, "psc")):
                    ps[g] = pb_ps.tile([128, 128], F32, tag=tag, name=tag)

                # bias broadcast: one full-partition start=True MM per bank
                # (E4.T @ bias_rows == bias in X layout; rank-4 trick)
                for g in (GR, GZ, GC):
                    nc.tensor.matmul(ps[g], e4_sb, bias_t(g),
                                     start=True, stop=False,
                                     skip_group_check=True)
                # x-side rounds (independent of h)
                for g in (GR, GZ, GC):
                    for r4 in range(4):
                        for c in range(2):
                            nc.tensor.matmul(
                                ps[g][32 * r4:32 * r4 + 32, :],
                                xct[:, c * N:(c + 1) * N],
                                wcat_t(g, c, r4),
                                start=False, stop=False,
                                tile_position=(0, 32 * r4),
                                skip_group_check=True)
                # recurrent rounds for r and z gates
                for g in (GR, GZ):
                    for r4 in range(4):
                        for k in range(4):
                            nc.tensor.matmul(
                                ps[g][32 * r4:32 * r4 + 32, :],
                                hdT[:, 32 * k:32 * k + 32],
                                u_t(g, k, r4),
                                start=False, stop=(k == 3),
                                tile_position=(0, 32 * r4),
                                skip_group_check=True)

                r_sb = pb.tile([128, 128], F32, tag="r_sb")
                nc.scalar.activation(r_sb, ps[GR], AF.Sigmoid)
                z_sb = pb.tile([128, 128], F32, tag="z_sb")
                nc.scalar.activation(z_sb, ps[GZ], AF.Sigmoid)

                hr = pb.tile([128, 128], BF16, tag="hr")
                nc.vector.tensor_tensor(hr, hd, r_sb, ALU.mult)
                hrT = pb.tile([128, 128], BF16, tag="hrT")
                nc.vector.transpose(hrT, hr)

                for r4 in range(4):
                    for k in range(4):
                        nc.tensor.matmul(
                            ps[GC][32 * r4:32 * r4 + 32, :],
                            hrT[:, 32 * k:32 * k + 32],
                            u_t(GC, k, r4),
                            start=False, stop=(k == 3),
                            tile_position=(0, 32 * r4),
                            skip_group_check=True)

                ht = pb.tile([128, 128], F32, tag="ht")
                nc.scalar.activation(ht, ps[GC], AF.Tanh)

                # h_new = hd + (z*mask) * (ht - hd)
                zeff = pb.tile([128, 128], F32, tag="zeff")
                nc.vector.tensor_scalar(zeff, z_sb, msc_sb[:, ds(iv, 1)], None, ALU.mult)
                dlt = pb.tile([128, 128], F32, tag="dlt")
                nc.vector.tensor_tensor(dlt, ht, hd, ALU.subtract)
                uu = pb.tile([128, 128], F32, tag="uu")
                nc.vector.tensor_tensor(uu, zeff, dlt, ALU.mult)
                nc.vector.tensor_tensor(h_state, hd, uu, ALU.add)

            with tc.For_i(0, t_steps, scan_unroll,
                          hint_engines=(mybir.EngineType.PE,)) as iv0:
                for u in range(scan_unroll):
                    scan_body(iv0 + u)

        # ================= Head =================
        with tc.tile_pool(name="ph", bufs=2) as ph, \
                tc.tile_pool(name="ph_ps", bufs=2, space="PSUM") as ph_ps:
            hbn0 = ph.tile([128, 128], F32, tag="hbn0")
            nc.vector.tensor_tensor(hbn0, h_state, bnw_sb, ALU.mult)
            hbn = ph.tile([128, 128], F32, tag="hbn")
            nc.vector.tensor_tensor(hbn, hbn0, bnb_sb, ALU.add)
            nc.sync.dma_start(hbn_st, hbn)

            hbnT = ph.tile([128, 128], F32, tag="hbnT")
            nc.vector.transpose(hbnT, hbn)
            psd = ph_ps.tile([N, O], F32, tag="psd")
            for k in range(4):
                nc.tensor.matmul(psd, hbnT[:, 32 * k:32 * k + 32],
                                 decw_sb[:, k * O:(k + 1) * O],
                                 start=(k == 0), stop=False)
            nc.tensor.matmul(psd, ones_sb[:, :N], decb_sb,
                             start=False, stop=True)

            negmax = ph.tile([N, 1], F32, tag="negmax")
            nc.vector.tensor_reduce(negmax, psd, mybir.AxisListType.X, ALU.max,
                                    negate=True)
            esum = ph.tile([N, 1], F32, tag="esum")
            etile = ph.tile([N, O], F32, tag="etile")
            nc.scalar.activation(etile, psd, AF.Exp, bias=negmax, accum_out=esum)
            lns = ph.tile([N, 1], F32, tag="lns")
            nc.scalar.activation(lns, esum, AF.Ln)
            nlns = ph.tile([N, 1], F32, tag="nlns")
            nc.vector.tensor_scalar(nlns, lns, -1.0, None, ALU.mult)
            lp = ph.tile([N, O], F32, tag="lp")
            nc.vector.tensor_scalar(lp, psd, negmax, nlns, ALU.add, ALU.add)
            nc.sync.dma_start(out_logp, lp)


# ---------------- host side ----------------

def _perm_cols(W, r):
    cols = np.concatenate(
        [np.arange(128 * k + 32 * r, 128 * k + 32 * r + 32) for k in range(4)])
    return np.ascontiguousarray(W[:, cols])


def _to_x_bcast(v):
    """per-h vector [H] -> X-layout [128,128] tile (same for every b)"""
    out = np.zeros((128, 128), np.float32)
    for r in range(4):
        for k in range(4):
            out[32 * r:32 * r + 32, 32 * k:32 * k + 32] = \
                v[128 * k + 32 * r:128 * k + 32 * r + 32][None, :]
    return out


def prep_host_inputs(inputs, t_steps=T, ncores=NCORES):
    """Build per-core in_maps (numpy only: slicing/layout/permutation)."""
    f32 = np.float32
    x = np.asarray(inputs["x"], f32)
    delta = np.asarray(inputs["delta"], f32)
    m = np.asarray(inputs["m"], f32)
    xf = np.asarray(inputs["x_forward"], f32)
    bs = np.asarray(inputs["batch_sizes"])
    n_tiles = t_steps // 4

    W_r, U_r, V_r, b_r = (np.asarray(inputs[k], f32) for k in ("W_r", "U_r", "V_r", "b_r"))
    W_z, U_z, V_z, b_z = (np.asarray(inputs[k], f32) for k in ("W_z", "U_z", "V_z", "b_z"))
    W, U, V, b = (np.asarray(inputs[k], f32) for k in ("W", "U", "V", "b"))
    Wgx, bgx = np.asarray(inputs["W_gamma_x"], f32), np.asarray(inputs["b_gamma_x"], f32)
    Wgh, bgh = np.asarray(inputs["W_gamma_h"], f32), np.asarray(inputs["b_gamma_h"], f32)
    decW, decb = np.asarray(inputs["dec_W"], f32), np.asarray(inputs["dec_b"], f32)
    bnw, bnb = np.asarray(inputs["bn_w"], f32), np.asarray(inputs["bn_b"], f32)

    gates = [(W_z, V_z, U_z, b_z), (W_r, V_r, U_r, b_r), (W, V, U, b)]
    wcat_perm = np.zeros((3, 2, 4, 128, 128), f32)
    u_perm = np.zeros((3, 4, 4, 128, 128), f32)
    bias_perm = np.zeros((3, 4, 1, 128), f32)
    for g, (Wg_, Vg_, Ug_, bg_) in enumerate(gates):
        Wc = np.vstack([Wg_, Vg_])  # [256, H]
        for r in range(4):
            pc = _perm_cols(Wc, r)
            wcat_perm[g, 0, r] = pc[:128]
            wcat_perm[g, 1, r] = pc[128:]
            bias_perm[g, r, 0] = _perm_cols(bg_[None, :], r)[0]
            pu = _perm_cols(Ug_, r)
            for k in range(4):
                u_perm[g, k, r] = pu[128 * k:128 * k + 128]

    bn_scale = bnw / np.sqrt(1.0 + BN_EPS)
    decw_packed = np.zeros((128, 4 * O), f32)
    for k in range(4):
        decw_packed[:, k * O:(k + 1) * O] = decW[128 * k:128 * k + 128, :]

    import ml_dtypes
    bf16 = ml_dtypes.bfloat16
    shared = {
        "wgx_rep": np.tile(Wgx[None, :], (128, 1)).astype(f32),
        "bgx_rep": np.tile(bgx[None, :], (128, 1)).astype(f32),
        "wg_neg": (-Wgh).astype(bf16),
        "g_bias_row": (Wgh.sum(0) + bgh)[None, :].astype(bf16),
        "identity": np.eye(128, dtype=f32),
        "e4": np.repeat(np.eye(4, dtype=bf16), 32, axis=1),
        "ones_col": np.ones((1, 128), f32),
        "ones_bf": np.ones((1, 128), bf16),
        "wcat_perm": wcat_perm.astype(bf16),
        "u_perm": u_perm.astype(bf16),
        "bias_perm": bias_perm.astype(bf16),
        "bnw_x": _to_x_bcast(bn_scale),
        "bnb_x": _to_x_bcast(bnb),
        "decw": decw_packed,
        "decb_row": decb[None, :].astype(f32),
    }

    in_maps = []
    n = B // ncores
    for c in range(ncores):
        sl = slice(c * n, (c + 1) * n)
        rows = np.arange(c * n, (c + 1) * n)
        masks = (rows[None, :] < bs[:t_steps, None]).astype(f32)  # [T, n]
        masks_ph = np.zeros((128, n_tiles), f32)
        masks_sc = np.zeros((128, t_steps), f32)
        for tp in range(4):
            masks_ph[32 * tp:32 * tp + 32, :] = masks[np.arange(n_tiles) * 4 + tp].T
        for r in range(4):
            masks_sc[32 * r:32 * r + 32, :] = masks.T
        im = dict(shared)
        im.update({
            "x_in": np.ascontiguousarray(x[sl, :t_steps]),
            "delta_in": np.ascontiguousarray(delta[sl, :t_steps]),
            "m_in": np.ascontiguousarray(m[sl, :t_steps]),
            "xf_in": np.ascontiguousarray(xf[sl, :t_steps]),
            "masks_ph": masks_ph,
            "masks_sc": masks_sc,
        })
        in_maps.append(im)
    return in_maps


_CACHED = {}


def _build(t_steps=T, scan_unroll=4, phasea_unroll=4):
    key = (t_steps, scan_unroll, phasea_unroll)
    if key not in _CACHED:
        nc = bacc.Bacc("TRN2", target_bir_lowering=False, debug=False)
        build_kernel(nc, t_steps=t_steps, scan_unroll=scan_unroll,
                     phasea_unroll=phasea_unroll)
        nc.compile()
        _CACHED[key] = nc
    return _CACHED[key]


def kernel(**inputs):
    nc = _build()
    in_maps = prep_host_inputs(inputs)
    res = run_bass_kernel_spmd(nc, in_maps, core_ids=list(range(NCORES)))
    outs = res.results
    logp = np.concatenate([outs[c]["out_logp"] for c in range(NCORES)], axis=0)
    hbn = np.concatenate([outs[c]["out_hbn"] for c in range(NCORES)], axis=0)
    return logp.astype(np.float32), hbn.astype(np.float32)
